# revision 65
# baseline (speedup 1.0000x reference)
"""Bass/Trainium2 kernel for nn_MultiHeadAttention (B=4, S=2048, E=512, H=8, dk=dv=8).

Sharding: 8 cores = (batch b, head-half hh).  Core 2b+hh computes causal
attention for batch b over heads [4hh, 4hh+4) for all 2048 queries, applies
its half of the output projection, and returns a partial output transposed
[E, S].  Host sums the two partials per batch, transposes, and adds bo.

Device layout notes (v2 — bf16 matmul path):
  - Host feeds query/key/value TRANSPOSED ([E, S]) and cast to bf16 so
    projections use them directly as matmul operands at 1 cycle/row.
  - Projection weights are host-packed bf16 "spread" layouts: Q/K outputs
    land at partitions {32h+d}; V outputs at columns {9h+d} with a ones
    column per head at 9h+0 accumulating the softmax denominator.
  - Scores are computed transposed ([t, q]); exp outputs bf16 tiles that
    feed the A@V matmul as the moving operand (V stationary).  The causal
    mask is applied AFTER exp as a bf16 0/1 multiply (keeps the
    scores->exp chain free of DVE hops).
  - 4 heads run concurrently on the PE array: scores via row-tiling
    (tile_position=(32h, 0), K=8) and A@V via col-tiling
    (tile_position=(0, 32h), M=9).
  - Softmax normalization per q-chunk: one full-tile DVE reciprocal, a
    bf16 rank-1 matmul partition-broadcast, one full-tile multiply; the
    last chunk's normalization is column-split to shorten the tail.
  - Emission is software-pipelined around the two co-critical engines
    (PE, power-throttled to ~50% duty, and the Activation engine's exp):
    scores(tb+1) are emitted before AV(tb); projections / normalization /
    out-projection are diced into <=0.6us closures drained via a work
    FIFO, one or two per t-block slot, with chunk boundaries prefetching
    the next chunk's first scores ahead of the final AV batch.
"""

import math

import numpy as np

B, S, E, H = 4, 2048, 512, 8
DK_H = DV_H = 8
NCORES = 8
HPC = H // 2  # heads per core = 4
SCALE = 1.0 / math.sqrt(DK_H)
NEG = -1.0e30
NQC = S // 512  # q chunks of 512
NTB = S // 128  # t blocks of 128
ECH = E // 128  # e chunks of 128

_cache: dict = {}


def _apply_tile_patch():
    """walrus in this image allows only one sync-wait per Drain; split the
    TileContext tail drain's waits across a chain of drains."""
    import concourse.mybir as mybir
    from concourse import tile
    from concourse.vector_clock import ScopedClock

    if getattr(tile.TileContext._drain_and_barrier, "_split_patch", False):
        return

    def _drain_and_barrier_split(self, tick_clock, wait_clock):
        drain_inst = self.nc.sync.drain()
        wait_clock.add_sem_waits(
            drain_inst.ins, ScopedClock({None: tick_clock.global_clock})
        )
        si = drain_inst.ins.sync_info
        if si is not None and si.on_wait and len(si.on_wait) > 1:
            waits = list(si.on_wait)
            si.on_wait = waits[:1]
            for entry in waits[1:]:
                extra = self.nc.sync.drain()
                extra.ins.sync_info = mybir.SyncInfo(on_wait=[entry], on_update=[])
        self.nc.all_engine_barrier()
        assert self.sems is not None
        popped = self.nc._tile_sem_poison_stack.pop()
        assert popped is self._sem_poison
        self.nc.clear_and_free_semaphores(list(self.sems.allocated().values()))
        self.nc.all_engine_barrier()

    _drain_and_barrier_split._split_patch = True
    tile.TileContext._drain_and_barrier = _drain_and_barrier_split


def _split_multi_waits(nc):
    """walrus in this image allows only one sync-wait per instruction;
    move excess waits onto single-wait NOPs inserted just before."""
    import concourse.mybir as mybir

    for blk in nc.m.functions[0].blocks:
        out = []
        for inst in blk.instructions:
            si = getattr(inst, "sync_info", None)
            if si is not None and si.on_wait and len(si.on_wait) > 1:
                waits = list(si.on_wait)
                for i, entry in enumerate(waits[:-1]):
                    out.append(
                        mybir.InstNoOp(
                            name=f"{inst.name}_w{i}",
                            engine=inst.engine,
                            ins=[],
                            outs=[],
                            bass_nofuse=True,
                            sync_info=mybir.SyncInfo(
                                on_wait=[entry], on_update=[]
                            ),
                        )
                    )
                si.on_wait = waits[-1:]
            out.append(inst)
        blk.instructions = out


def _build():
    import concourse.bass as bassmod
    import concourse.mybir as mybir
    from concourse import tile

    _apply_tile_patch()
    f32 = mybir.dt.float32
    bf16 = mybir.dt.bfloat16
    Exp = mybir.ActivationFunctionType.Exp

    nc = bassmod.Bass()
    qT = nc.declare_dram_parameter("qT", [E, S], bf16, isOutput=False)
    kT = nc.declare_dram_parameter("kT", [E, S], bf16, isOutput=False)
    vT = nc.declare_dram_parameter("vT", [E, S], bf16, isOutput=False)
    # weights host-packed partition-major so their DMAs are contiguous
    wq = nc.declare_dram_parameter("wq", [128, ECH * 128], bf16, isOutput=False)
    wk = nc.declare_dram_parameter("wk", [128, ECH * 128], bf16, isOutput=False)
    wv = nc.declare_dram_parameter("wv", [128, ECH * HPC * 9], bf16,
                                   isOutput=False)
    wo = nc.declare_dram_parameter("wo", [128, E], bf16, isOutput=False)
    msk = nc.declare_dram_parameter("msk", [128, 2 * 128], bf16, isOutput=False)
    out = nc.declare_dram_parameter("out", [E, S], f32, isOutput=True)

    with tile.TileContext(nc) as tc:
        with (
            tc.tile_pool(name="singles", bufs=1) as singles,
            tc.tile_pool(name="loads", bufs=12) as loads,
            tc.tile_pool(name="abuf", bufs=12) as abuf,
            tc.tile_pool(name="outs", bufs=8) as outs,
            tc.tile_pool(name="recips", bufs=8) as recips,
            tc.tile_pool(name="onorms", bufs=3) as onorms,
            tc.tile_pool(name="ps_sc", bufs=2, space="PSUM") as ps_sc,
            tc.tile_pool(name="ps_av", bufs=2, space="PSUM") as ps_av,
            tc.tile_pool(name="ps_misc", bufs=2, space="PSUM") as ps_misc,
        ):
            # ---- resident tensors -------------------------------------
            wq_sb = singles.tile([128, ECH, 128], bf16, tag="wq")
            wk_sb = singles.tile([128, ECH, 128], bf16, tag="wk")
            wv_sb = singles.tile([128, ECH, HPC * 9], bf16, tag="wv")
            wo_sb = singles.tile([128, ECH, 128], bf16, tag="wo")
            msk_sb = singles.tile([128, 2, 128], bf16, tag="msk")
            # startup: K path fully on the Sync DMA queue, Q path (incl. its
            # weight) on GpSimd, so both projection chains run in parallel;
            # wv/wo/msk are deferred until after the q tiles are queued
            nc.gpsimd.dma_start(out=wq_sb, in_=wq.rearrange("p (c m) -> p c m", c=ECH))
            nc.sync.dma_start(out=wk_sb, in_=wk.rearrange("p (c m) -> p c m", c=ECH))

            def late_weights():
                nc.gpsimd.dma_start(
                    out=wv_sb, in_=wv.rearrange("p (c m) -> p c m", c=ECH)
                )
                nc.gpsimd.dma_start(
                    out=wo_sb, in_=wo.rearrange("p (c m) -> p c m", c=ECH)
                )
                nc.gpsimd.dma_start(
                    out=msk_sb, in_=msk.rearrange("p (g n) -> p g n", g=2)
                )

            # per-chunk projected tensors (separate tiles so the tile
            # dependency tracker never serializes chunk c's reads against
            # chunk c+2's writes)
            KT_t = [
                singles.tile([128, 512], bf16, tag=f"KT{c}", name=f"KT{c}")
                for c in range(NQC)
            ]
            QT_t = [
                singles.tile([128, 512], bf16, tag=f"QT{c}", name=f"QT{c}")
                for c in range(NQC)
            ]
            V_t = [
                singles.tile([128, 4, HPC, 9], bf16, tag=f"V{c}", name=f"V{c}")
                for c in range(NQC)
            ]

            ones9 = singles.tile([128, 9], bf16, tag="ones9")
            nc.vector.memset(ones9, 1.0)
            for c in range(NQC):
                nc.vector.memset(V_t[c][:, :, :, 0:1], 1.0)
            # preload the exp activation table while DMAs run
            warm = abuf.tile([128, 2], bf16, tag="warm")
            nc.scalar.activation(warm, ones9[:, 0:2], Exp, scale=1.0)

            dmaq = [nc.sync, nc.gpsimd]

            def proj_pieces(c):
                """Q/K/V projections for chunk c as dicts of single-matmul
                emission closures (<=0.6us of tensor work each) so pieces
                fit a slot's tensor headroom without starving the exp
                pipeline."""
                cs = slice(c * 512, (c + 1) * 512)
                st = {}

                def dma_in(src, key, n):
                    tiles = []
                    for e in range(ECH):
                        if c == 0 and key in ("k", "q"):
                            q = dmaq[0 if key == "k" else 1]
                        else:
                            q = dmaq[(e + n) % 2]
                        t = loads.tile([128, 512], bf16, tag="ld",
                                       name=f"{key}{c}_{e}")
                        q.dma_start(
                            out=t, in_=src[e * 128:(e + 1) * 128, cs]
                        )
                        tiles.append(t)
                    st[key] = tiles

                def qk_mm(key, w_sb, dst, e):
                    if e == 0:
                        st[key + "ps"] = ps_misc.tile(
                            [128, 512], f32, tag="ps", name=f"{key}ps{c}"
                        )
                    nc.tensor.matmul(
                        st[key + "ps"], w_sb[:, e, :], st[key][e][:, :],
                        start=(e == 0), stop=(e == ECH - 1),
                    )
                    if e == ECH - 1:
                        # split the cast so the first score batch (heads
                        # 0-1, partitions < 64) gates on the first half
                        nc.vector.tensor_copy(
                            dst[0:64, :], st[key + "ps"][0:64, :]
                        )
                        nc.vector.tensor_copy(
                            dst[64:128, :], st[key + "ps"][64:128, :]
                        )

                def v_mm(tb, half):
                    if half == 0:
                        st[f"vps{tb}"] = ps_misc.tile(
                            [128, HPC * 9], f32, tag="ps", name=f"vps{c}_{tb}"
                        )
                    for e in (0, 1) if half == 0 else (2, 3):
                        nc.tensor.matmul(
                            st[f"vps{tb}"],
                            st["v"][e][:, tb * 128:(tb + 1) * 128],
                            wv_sb[:, e, :],
                            start=(e == 0), stop=(e == ECH - 1),
                        )
                    if half == 1:
                        dst = V_t[c][:, tb, :, 1:9]
                        src = st[f"vps{tb}"].rearrange(
                            "p (h n) -> p h n", n=9
                        )[:, :, 1:9]
                        nc.vector.tensor_copy(dst, src)

                return {
                    "q": [
                        lambda e=e: (
                            dma_in(qT, "q", 0) if e == 0 else None,
                            qk_mm("q", wq_sb, QT_t[c], e),
                        )
                        for e in range(ECH)
                    ],
                    "k": [
                        lambda e=e: (
                            dma_in(kT, "k", 1) if e == 0 else None,
                            qk_mm("k", wk_sb, KT_t[c], e),
                        )
                        for e in range(ECH)
                    ],
                    "v": [
                        lambda tb=tb, half=half: (
                            dma_in(vT, "v", 0)
                            if (tb == 0 and half == 0) else None,
                            v_mm(tb, half),
                        )
                        for tb in range(4)
                        for half in range(2)
                    ],
                }

            st_norm = {}

            def norm_recip(c, av, col):
                """Reciprocal+cast for chunk c columns [col] (DVE)."""
                recip_bf = st_norm[(c, "rb", col.start)] = recips.tile(
                    [128, 512], bf16, tag="rb", name=f"rb{c}_{col.start}"
                )
                recip_t = recips.tile([128, 512], f32, tag="rc",
                                      name=f"rc{c}_{col.start}")
                nc.vector.reciprocal(recip_t[:, col], av[:, col])
                nc.vector.tensor_copy(recip_bf[:, col], recip_t[:, col])

            def norm_mul(c, av, col):
                """rank-1 broadcast + normalize-multiply for columns [col]."""
                recip_bf = st_norm[(c, "rb", col.start)]
                rep_ps = ps_misc.tile([128, 512], f32, tag="ps",
                                      name=f"rep{c}_{col.start}")
                for h in range(HPC):
                    nc.tensor.matmul(
                        rep_ps[32 * h:32 * h + 9, col],
                        ones9[32 * h:32 * h + 1, :],
                        recip_bf[32 * h:32 * h + 1, col],
                        start=True, stop=True,
                        tile_position=(32 * h, 32 * h),
                    )
                rep_sb = recips.tile([128, 512], f32, tag="rs",
                                     name=f"rs{c}_{col.start}")
                nc.vector.tensor_copy(rep_sb[:, col], rep_ps[:, col])
                onorm_c = st_norm.get((c, "on"))
                if onorm_c is None:
                    onorm_c = st_norm[(c, "on")] = onorms.tile(
                        [128, 512], bf16, tag="on", name=f"on{c}"
                    )
                nc.vector.tensor_mul(onorm_c[:, col], av[:, col], rep_sb[:, col])
                return onorm_c

            def outproj_piece(c, es, col=slice(0, 512)):
                cs = slice(c * 512 + col.start, c * 512 + col.stop)
                onorm_c = st_norm[(c, "on")]
                for e in es:
                    f_ps = ps_misc.tile([128, 512], f32, tag="ps",
                                        name=f"fps{c}_{e}_{col.start}")
                    nc.tensor.matmul(
                        f_ps[:, col], wo_sb[:, e, :], onorm_c[:, col],
                        start=True, stop=True,
                    )
                    fsb = outs.tile([128, 512], f32, tag="f",
                                    name=f"f{c}_{e}_{col.start}")
                    nc.vector.tensor_copy(fsb[:, col], f_ps[:, col])
                    nc.gpsimd.dma_start(
                        out=out[e * 128:(e + 1) * 128, cs], in_=fsb[:, col]
                    )

            FULL = slice(0, 512)

            # ---- attention, software-pipelined ------------------------
            def emit_sa(c, tb):
                """Scores + mask + exp for (chunk c, t-block tb)."""
                d = 128 * tb - 512 * c  # diagonal offset within chunk
                vstart = max(d, 0)
                scs = [
                    ps_sc.tile([128, 2, 512], f32, tag="sc",
                               name=f"sc{c}_{tb}_{g}")
                    for g in range(2)
                ]
                ats = [
                    abuf.tile([128, 2, 512], bf16, tag="a",
                              name=f"a{c}_{tb}_{g}")
                    for g in range(2)
                ]
                for h in range(HPC):
                    g, j = divmod(h, 2)
                    nc.tensor.matmul(
                        scs[g][:, j, vstart:512],
                        KT_t[tb // 4][32 * h:32 * h + 8,
                                      (tb % 4) * 128:(tb % 4 + 1) * 128],
                        QT_t[c][32 * h:32 * h + 8, vstart:512],
                        start=True, stop=True,
                        tile_position=(32 * h, 0),
                    )
                for g in range(2):
                    nc.scalar.activation(
                        ats[g][:, :, vstart:512], scs[g][:, :, vstart:512],
                        Exp, scale=SCALE,
                    )
                    if d >= 0:
                        # zero the upper triangle AFTER exp (bf16 SBUF mul
                        # runs in the DVE 2x mode and keeps the scores->exp
                        # chain free of DVE hops)
                        nc.vector.tensor_mul(
                            ats[g][:, :, d:d + 128],
                            ats[g][:, :, d:d + 128],
                            msk_sb,
                        )
                return ats, vstart

            # prefix: chunk 0's K/Q projections inline, first scores/exp,
            # then chunk 0's first V block and chunk 1's Q/K — all during
            # the act-idle startup window
            sa_pre = {}  # (c, tb) -> (ats, vstart) emitted ahead of its chunk
            p0 = proj_pieces(0)
            pp = {c: proj_pieces(c) for c in range(1, NQC)}
            for f in p0["k"]:
                f()
            for f in p0["q"]:
                f()
            late_weights()
            sa_pre[(0, 0)] = emit_sa(0, 0)
            p0["v"][0]()
            p0["v"][1]()
            for f in pp[1]["q"]:
                f()
            for f in pp[1]["k"]:
                f()

            def out_pieces(c):
                return [
                    lambda pc=c, e=e: outproj_piece(pc, (e,))
                    for e in range(ECH)
                ]

            work = []  # FIFO of deferred emission closures
            avs = {}
            for c in range(NQC):
                av = avs[c] = ps_av.tile([128, 512], f32, tag="av",
                                         name=f"av{c}")
                nc.vector.memset(av, 0.0)
                ntb = 4 * (c + 1)

                if c == 0:
                    work.extend(p0["v"][2:8])  # V1-3(0)
                else:
                    if c >= 2:
                        work.extend(pp[c]["k"])
                        work.extend(out_pieces(c - 2))
                    work.extend(pp[c]["v"])
                    pav = avs[c - 1]
                    work.append(
                        lambda pc=c - 1, pav=pav: norm_recip(pc, pav, FULL)
                    )
                    if c + 1 < NQC:
                        work.extend(pp[c + 1]["q"])
                    else:
                        work.extend([None, None, None])
                    work.append(
                        lambda pc=c - 1, pav=pav: norm_mul(pc, pav, FULL)
                    )
                    if c == NQC - 1:
                        work.extend(out_pieces(c - 1))

                pend = None  # (tb, ats, vstart) waiting for its AV matmuls
                for tb in range(ntb):
                    if (c, tb) in sa_pre:
                        ats, vstart = sa_pre.pop((c, tb))
                    else:
                        ats, vstart = emit_sa(c, tb)
                    # AV for the PREVIOUS tb — keeps exp ahead of the PE
                    if pend is not None:
                        ptb, pats, pvs = pend
                        for h in range(HPC):
                            g, j = divmod(h, 2)
                            nc.tensor.matmul(
                                av[32 * h:32 * h + 9, pvs:512],
                                V_t[ptb // 4][:, ptb % 4, h, :],
                                pats[g][:, j, pvs:512],
                                start=(ptb == 0), stop=False,
                                tile_position=(0, 32 * h),
                            )
                    pend = (tb, ats, vstart)
                    # drain the FIFO evenly across this chunk's slots,
                    # holding back the first 2 slots (c>0) so the boundary
                    # scores are never stuck behind pieces
                    hold = 0 if c == 0 else 2
                    if tb >= hold:
                        npop = -(-len(work) // (ntb - tb))  # ceil
                        for _ in range(npop):
                            piece = work.pop(0)
                            if piece is not None:
                                piece()
                    if c == NQC - 1 and tb == ntb - 2:
                        # av cols [0:256] are final once AV(ntb-3) is
                        # emitted (this slot) — start the tail's first-half
                        # reciprocal two slots early
                        norm_recip(c, av, slice(0, 256))
                    elif c == NQC - 1 and tb == ntb - 1:
                        norm_mul(c, av, slice(0, 256))
                # prefetch the next chunk's first TWO score/exp groups
                # ahead of the final AV batch so the act pipeline never
                # drains at the boundary (the second group would otherwise
                # queue behind the final-AV matmuls)
                if c + 1 < NQC:
                    sa_pre[(c + 1, 0)] = emit_sa(c + 1, 0)
                    sa_pre[(c + 1, 1)] = emit_sa(c + 1, 1)
                ptb, pats, pvs = pend
                for h in range(HPC):
                    g, j = divmod(h, 2)
                    nc.tensor.matmul(
                        av[32 * h:32 * h + 9, pvs:512],
                        V_t[ptb // 4][:, ptb % 4, h, :],
                        pats[g][:, j, pvs:512],
                        start=(ptb == 0), stop=True,
                        tile_position=(0, 32 * h),
                    )

            # ---- tail: last chunk's norm + outproj, column-split; the
            # second half's reciprocal (DVE) overlaps the first half's
            # out-projection matmuls
            lc = NQC - 1
            lav = avs[lc]
            H1, H2 = slice(0, 256), slice(256, 512)
            norm_recip(lc, lav, H2)
            outproj_piece(lc, range(ECH), H1)
            norm_mul(lc, lav, H2)
            outproj_piece(lc, range(ECH), H2)
    _split_multi_waits(nc)
    return nc


def _prep_inputs(query, key, value, Wq, Wk, Wv, Wo):
    """Build the 8 per-core input maps (host-side sharding/layout)."""
    import ml_dtypes

    bf16 = np.dtype(ml_dtypes.bfloat16)
    qTs = [np.ascontiguousarray(query[b].T).astype(bf16) for b in range(B)]
    kTs = [np.ascontiguousarray(key[b].T).astype(bf16) for b in range(B)]
    vTs = [np.ascontiguousarray(value[b].T).astype(bf16) for b in range(B)]

    mask = np.where(
        np.arange(128)[:, None] <= np.arange(128)[None, :], 1.0, 0.0
    ).astype(np.float32)
    msk2 = np.ascontiguousarray(np.tile(mask, (1, 2))).astype(bf16)

    in_maps = []
    for core in range(NCORES):
        b, hh = divmod(core, 2)
        wq_p = np.zeros((E, 128), np.float32)
        wk_p = np.zeros((E, 128), np.float32)
        wv_p = np.zeros((E, HPC * 9), np.float32)
        wo_p = np.zeros((128, E), np.float32)
        for h in range(HPC):
            g = 4 * hh + h
            wq_p[:, 32 * h:32 * h + 8] = Wq[g]
            wk_p[:, 32 * h:32 * h + 8] = Wk[g]
            wv_p[:, 9 * h + 1:9 * h + 9] = Wv[g]
            wo_p[32 * h + 1:32 * h + 9, :] = Wo[8 * g:8 * g + 8, :]
        def pack(w):  # [E, M] -> [128, ECH*M] partition-major
            m = w.shape[1]
            return np.ascontiguousarray(
                w.reshape(ECH, 128, m).transpose(1, 0, 2).reshape(128, ECH * m)
            )

        in_maps.append(
            {
                "qT": qTs[b], "kT": kTs[b], "vT": vTs[b],
                "wq": pack(wq_p).astype(bf16), "wk": pack(wk_p).astype(bf16),
                "wv": pack(wv_p).astype(bf16), "wo": wo_p.astype(bf16),
                "msk": msk2,
            }
        )
    return in_maps


def _reference_numpy(query, key, value, padding_mask, decoder_mask,
                     Wq, Wk, Wv, Wo, bo):
    """Fallback (non-default masks): plain numpy replica of the reference."""
    q = np.einsum("bse,hed->bhsd", query, Wq)
    k = np.einsum("bse,hed->bhsd", key, Wk)
    v = np.einsum("bse,hed->bhsd", value, Wv)
    s = np.einsum("bhsd,bhtd->bhst", q, k)
    if decoder_mask:
        tril = np.tril(s)
        s = np.where(tril == 0.0, -np.inf, s)
    s = np.where(padding_mask[:, None, :, :], s, -np.inf)
    s = s / np.sqrt(np.float32(DK_H))
    m = np.max(s, axis=-1, keepdims=True)
    e = np.exp(s - m)
    a = e / np.sum(e, axis=-1, keepdims=True)
    o = np.einsum("bhst,bhtd->bhsd", a, v)
    o = o.transpose(0, 2, 1, 3).reshape(o.shape[0], o.shape[2], H * DV_H)
    return (o @ Wo + bo).astype(np.float32)


def kernel(query, key, value, padding_mask, decoder_mask, Wq, Wk, Wv, Wo, bo,
           **run_kwargs):
    query = np.asarray(query, np.float32)
    key = np.asarray(key, np.float32)
    value = np.asarray(value, np.float32)
    Wq = np.asarray(Wq, np.float32)
    Wk = np.asarray(Wk, np.float32)
    Wv = np.asarray(Wv, np.float32)
    Wo = np.asarray(Wo, np.float32)
    bo = np.asarray(bo, np.float32)
    pm = np.asarray(padding_mask)
    dm = int(np.asarray(decoder_mask))

    if not bool(pm.all()) or not dm:
        return _reference_numpy(
            query, key, value, pm.astype(bool), dm, Wq, Wk, Wv, Wo, bo
        )

    from concourse.bass_utils import run_bass_kernel_spmd

    if "nc" not in _cache:
        _cache["nc"] = _build()
    nc = _cache["nc"]

    in_maps = _prep_inputs(query, key, value, Wq, Wk, Wv, Wo)
    res = run_bass_kernel_spmd(nc, in_maps, list(range(NCORES)), **run_kwargs)

    outp = np.empty((B, S, E), np.float32)
    for b in range(B):
        fT = res.results[2 * b]["out"] + res.results[2 * b + 1]["out"]
        outp[b] = fT.T + bo
    if run_kwargs:
        kernel.last_result = res
    return outp
